# revision 2
# baseline (speedup 1.0000x reference)
"""Trainium2 Bass kernel for nn_AbsoluteNeuralLayer.

Reference computation:
    classical = x @ classical_weights + classical_biases          # [B, DOUT]
    probs[j]  = |scan of circulant "rotations" applied to s0|[0]^2
    out       = tanh(classical + probs[None, :])

Key simplification: the scan state s0 is a constant vector, and every step
maps a constant vector to a constant vector scaled by cos(angle)
(s_new[i] = cos*s - sin*s + sin*s = cos*s elementwise).  Hence
    probs[j] = (prod_{t<48} cos(ang[j, t]))^2 / DIN
with ang[j, 3*d+g] = absolute_weights[d, j, g] for g < 3.

Sharding (8 cores): batch split 4 ways x dout split 2 ways.  Each core
computes out[1024 batch rows, 1024 dout cols] as tanh(x_s @ W_s + bias_s +
probs_s) with dout on PSUM partitions and batch on the moving free dim,
accumulating over K=2048 in 16 k-tiles.  All matmul operands are bf16
(tanh of a +-45-sigma preactivation swallows the rounding: measured rel
err ~1.2e-2 vs the 2e-2 gate), which halves HBM traffic vs fp32r and
makes the kernel purely PE-bound: 256 MMs x 512 cols ~= 54.6 us at the
2.4 GHz warm clock.  probs+bias are computed once per core on ACT/DVE
(tiny) and applied as the per-partition bias of the Tanh activation that
drains PSUM.  Outputs are written transposed (bf16) and un-transposed /
upcast on the host during the gather.

Schedule (8 PSUM banks):
  pass A: batch-chunk 0, k-outer x n-inner so compute tracks the k-major
          W stream; 16 warmup matmuls anchor the HAM activity window at
          t=0 so the PE un-throttles to 2.4 GHz ~while the first real
          data lands (~1.8 us in).
  pass B: batch-chunk 1, n-outer x k-inner with the bank's epilogue
          (ACT tanh + out DMA) inlined right after its stop-matmul; each
          epilogue overlaps the next bank's 3.4 us of matmuls, so only
          bank 7's epilogue lands in the kernel tail.
All inputs are host-packed into SBUF layout so every DMA is a single
contiguous-per-partition transfer.
"""

import math

import numpy as np
import ml_dtypes

import concourse.bacc as bacc
import concourse.mybir as mybir
from concourse.tile import TileContext
from concourse.bass_utils import run_bass_kernel_spmd

B, DIN, DOUT, DEPTH = 4096, 2048, 2048, 16
NCORES = 8
BB, DB = 4, 2            # batch blocks x dout blocks (BB*DB == NCORES)
MB, NB = B // BB, DOUT // DB   # per-core batch rows (1024) / dout cols (1024)
KT = DIN // 128          # 16 contraction tiles
NT = NB // 128           # 8 dout tiles
MCH = 512                # batch chunk = one PSUM bank of fp32
MC = MB // MCH           # 2 chunks
NANG = 3 * DEPTH         # 48 angles per output column

A_CHUNKS = [1, 1, 2, 2, 2, 2, 2, 2, 1, 1]   # pass-A k-stream granularity: small
                                            # first chunk so the first matmul
                                            # starts ~1.8 us in
B_CHUNKS = [4, 4, 4, 4]                  # x1 stream (behind pass A's bytes)
WARMUP_MMS = 16                          # PE warmup matmuls (HAM un-throttle)

F32 = mybir.dt.float32
BF16 = mybir.dt.bfloat16
AF = mybir.ActivationFunctionType

_NC_CACHE = None


def _chunk_offsets(chunks):
    off, out = 0, []
    for c in chunks:
        out.append((off, c))
        off += c
    return out


def _build():
    nc = bacc.Bacc("TRN2", target_bir_lowering=False, debug=False, num_devices=NCORES)
    # host-packed SBUF layouts:
    #   wb [p, k*NB + n]          = W[128k+p, n]
    #   xb [p, (u*KT + k)*MCH+m]  = x[u*MCH + m, 128k+p]   (u = m-chunk)
    wb = nc.dram_tensor("wb", [128, KT * NB], BF16, kind="ExternalInput")
    xb = nc.dram_tensor("xb", [128, MC * KT * MCH], BF16, kind="ExternalInput")
    ang = nc.dram_tensor("ang", [128, NT * NANG], F32, kind="ExternalInput")
    bias = nc.dram_tensor("bias", [128, NT], F32, kind="ExternalInput")
    outT = nc.dram_tensor("outT", [NB, MB], BF16, kind="ExternalOutput")

    with TileContext(nc) as tc:
        with (
            tc.tile_pool(name="big", bufs=1) as big,
            tc.tile_pool(name="small", bufs=1) as small,
            tc.tile_pool(name="outp", bufs=8) as outp,
            tc.tile_pool(name="psum", bufs=1, space="PSUM") as psump,
        ):
            # ---- pass-A stream: W (full) + x chunk 0, k-chunked ----
            wg = [None] * KT   # (tile, col offset) per k
            xs = [[None] * KT for _ in range(MC)]
            for ci, (k0, kn) in enumerate(_chunk_offsets(A_CHUNKS)):
                wt = big.tile([128, kn * NB], BF16, tag=f"w{ci}", name=f"w{ci}")
                nc.sync.dma_start(out=wt, in_=wb[:, k0 * NB:(k0 + kn) * NB])
                for i in range(kn):
                    wg[k0 + i] = (wt, i * NB)
                xt = big.tile([128, kn * MCH], BF16, tag=f"x0_{ci}", name=f"x0_{ci}")
                nc.sync.dma_start(out=xt, in_=xb[:, k0 * MCH:(k0 + kn) * MCH])
                for i in range(kn):
                    xs[0][k0 + i] = (xt, i * MCH)
                if ci == 0:
                    # ang + bias: tiny, needed only by the first epilogue
                    # (~28 us in); slot them right after the first k-chunk
                    ang_sb = small.tile([128, NT * NANG], F32, tag="ang")
                    nc.sync.dma_start(out=ang_sb, in_=ang[:, :])
                    bias_sb = small.tile([128, NT], F32, tag="bias")
                    nc.sync.dma_start(out=bias_sb, in_=bias[:, :])

            # ---- probs + bias compute (tiny, ACT/DVE) ----
            halfpi = small.tile([128, 1], F32, tag="halfpi")
            nc.any.memset(halfpi, math.pi / 2)
            cos_sb = small.tile([128, NT * NANG], F32, tag="cos")
            nc.scalar.activation(cos_sb, ang_sb, AF.Sin, bias=halfpi)

            def v3(t):
                return t.rearrange("p (a b) -> p a b", a=NT)

            t24 = small.tile([128, NT * 24], F32, tag="t24")
            nc.vector.tensor_mul(v3(t24), v3(cos_sb)[:, :, 0:24], v3(cos_sb)[:, :, 24:48])
            t12 = small.tile([128, NT * 12], F32, tag="t12")
            nc.vector.tensor_mul(v3(t12), v3(t24)[:, :, 0:12], v3(t24)[:, :, 12:24])
            t6 = small.tile([128, NT * 6], F32, tag="t6")
            nc.vector.tensor_mul(v3(t6), v3(t12)[:, :, 0:6], v3(t12)[:, :, 6:12])
            t3 = small.tile([128, NT * 3], F32, tag="t3")
            nc.vector.tensor_mul(v3(t3), v3(t6)[:, :, 0:3], v3(t6)[:, :, 3:6])
            t1 = small.tile([128, NT], F32, tag="t1")
            nc.vector.tensor_mul(v3(t1), v3(t3)[:, :, 0:1], v3(t3)[:, :, 1:2])
            nc.vector.tensor_mul(v3(t1), v3(t1), v3(t3)[:, :, 2:3])
            sq = small.tile([128, NT], F32, tag="sq")
            nc.vector.tensor_mul(sq, t1, t1)
            nc.vector.tensor_scalar_mul(sq, sq, 1.0 / DIN)
            btot = small.tile([128, NT], F32, tag="btot")
            nc.vector.tensor_add(btot, sq, bias_sb)

            def mm_w(k, n):
                wt, off = wg[k]
                return wt[:, off + 128 * n:off + 128 * (n + 1)]

            def mm_x(u, k):
                xt, off = xs[u][k]
                return xt[:, off:off + MCH]

            def epilogue(n, ps_tile, u, dma_eng):
                # ACT on scalar; out DMA issued from whichever HWDGE ring is
                # idle at that point (scalar during the input stream, sync
                # once the input stream has drained)
                o = outp.tile([128, MCH], BF16, tag="o", name=f"o{n}_{u}")
                nc.scalar.activation(o, ps_tile, AF.Tanh, bias=btot[:, n:n + 1])
                dma_eng.dma_start(
                    out=outT[128 * n:128 * (n + 1), u * MCH:(u + 1) * MCH], in_=o
                )

            # ---- pass A: m-chunk 0, k-outer over 8 PSUM groups ----
            psA = [
                psump.tile([128, MCH], F32, tag=f"ps{n}", name=f"psA{n}")
                for n in range(NT)
            ]
            # PE warmup: dependency-free matmuls into psA[0] flip the HAM
            # clock gate toward 8/8 while the first real data streams in, so
            # the real stream starts at 2.4 GHz instead of 1.2 GHz.
            warm = small.tile([128, 128], BF16, tag="warm")
            nc.any.memset(warm, 0.0)
            for i in range(WARMUP_MMS):
                nc.tensor.matmul(psA[0][:, 0:128], warm, warm, start=True, stop=True)
            for k in range(KT):
                for n in range(NT):
                    nc.tensor.matmul(
                        psA[n], mm_w(k, n), mm_x(0, k),
                        start=(k == 0), stop=(k == KT - 1),
                    )

            # x chunk 1 stream (issued here so the ring stays fed behind the
            # pass-A bytes without competing with them)
            for ci, (k0, kn) in enumerate(_chunk_offsets(B_CHUNKS)):
                xt = big.tile([128, kn * MCH], BF16, tag=f"x1_{ci}", name=f"x1_{ci}")
                nc.sync.dma_start(
                    out=xt, in_=xb[:, (KT + k0) * MCH:(KT + k0 + kn) * MCH]
                )
                for i in range(kn):
                    xs[1][k0 + i] = (xt, i * MCH)

            # pass A epilogues (ACT) — free banks in n order for pass B
            for n in range(NT):
                epilogue(n, psA[n], 0, nc.scalar)

            # ---- pass B: m-chunk 1, n-outer with inline epilogues ----
            for n in range(NT):
                psB = psump.tile([128, MCH], F32, tag=f"ps{n}", name=f"psB{n}")
                for k in range(KT):
                    nc.tensor.matmul(
                        psB, mm_w(k, n), mm_x(1, k),
                        start=(k == 0), stop=(k == KT - 1),
                    )
                epilogue(n, psB, 1, nc.sync)

    nc.compile()
    return nc


def _get_nc():
    global _NC_CACHE
    if _NC_CACHE is None:
        _NC_CACHE = _build()
    return _NC_CACHE


def _in_map_for_core(core, x, absolute_weights, classical_weights, classical_biases):
    i, j = core % BB, core // BB
    rows = slice(i * MB, (i + 1) * MB)
    cols = slice(j * NB, (j + 1) * NB)
    # wb[p, k*NB + n] = W[128k+p, n]
    wbm = np.ascontiguousarray(
        classical_weights[:, cols].reshape(KT, 128, NB).transpose(1, 0, 2).reshape(128, KT * NB)
    )
    # xb[p, (u*KT + k)*MCH + m] = x[rows][u*MCH+m, 128k+p]
    xsT = x[rows, :].T                                        # [DIN, MB] view
    xr = xsT.reshape(KT, 128, MC, MCH)                        # [k, p, u, m]
    xbm = np.ascontiguousarray(xr.transpose(1, 2, 0, 3).reshape(128, MC * KT * MCH))
    # ang[j_local, 3*d+g] = absolute_weights[d, j, g]
    angj = np.transpose(absolute_weights[:, cols, :3], (1, 0, 2)).reshape(NB, NANG)
    ang_sb = np.ascontiguousarray(
        angj.reshape(NT, 128, NANG).transpose(1, 0, 2).reshape(128, NT * NANG)
    )
    bias_sb = np.ascontiguousarray(classical_biases[cols].reshape(NT, 128).T)
    return {
        "wb": wbm.astype(ml_dtypes.bfloat16),
        "xb": xbm.astype(ml_dtypes.bfloat16),
        "ang": ang_sb.astype(np.float32, copy=False),
        "bias": bias_sb.astype(np.float32, copy=False),
    }


def kernel(x, absolute_weights, classical_weights, classical_biases, **_ignored):
    x = np.asarray(x, dtype=np.float32)
    absolute_weights = np.asarray(absolute_weights, dtype=np.float32)
    classical_weights = np.asarray(classical_weights, dtype=np.float32)
    classical_biases = np.asarray(classical_biases, dtype=np.float32)

    nc = _get_nc()
    in_maps = [
        _in_map_for_core(c, x, absolute_weights, classical_weights, classical_biases)
        for c in range(NCORES)
    ]
    res = run_bass_kernel_spmd(nc, in_maps, list(range(NCORES)))

    out = np.empty((B, DOUT), np.float32)
    for c in range(NCORES):
        i, j = c % BB, c // BB
        out[i * MB:(i + 1) * MB, j * NB:(j + 1) * NB] = (
            res.results[c]["outT"].astype(np.float32).T
        )
    return out
